# revision 14
# baseline (speedup 1.0000x reference)
"""Causal self-attention on 8 trn2 NeuronCores.

Sharding: core c -> (batch b = c//2, head-group hg = c%2 of 8 heads).
Each core computes, for its batch and its 8 heads:
  qT,kT = (x[b] @ Wqk_shard).T        (q pre-scaled by 1/sqrt(hd))
  V     = x[b] @ Wv_shard
  S^T   = kT_h.T @ qT_h  per head     (s on partitions, t on free dim)
  P^T   = exp(S^T) with causal mask   (no max-subtraction: logits are O(5))
  yT    = V_aug.T @ P^T               (V carries a ones column -> row 64 = softmax denom)
  out_partial = y_local @ Wout_rows   ([T, E] fp32 partial sum)
Host: out[b] = partial[2b] + partial[2b+1] + b_out.

All matmul inputs fp16, PSUM accumulation fp32. x is pre-transposed and
pre-cast on host so no on-chip transpose is needed.
"""

import numpy as np

B, T, E, H, HD = 4, 2048, 1024, 16, 64
HPC = 8            # heads per core
DL = HPC * HD      # 512 local y dims per core
NT = T // 512      # 4 t-chunks of 512
NS = T // 128      # 16 s-tiles of 128
NE = E // 128      # 8 e-tiles

_CACHE = {}


def _make_tc_class():
    """TileContext whose tail drain splits sem waits across single-wait NOPs.

    The walrus build in this container rejects instructions carrying more
    than a couple of sync waits ("Too many sync wait commands" on the Tile
    tail Drain), so emit one NOP per logical proc, each with one wait.
    """
    import concourse.tile as tile
    from concourse.vector_clock import ScopedClock, VectorClock

    class TC(tile.TileContext):
        def _drain_and_barrier(self, tick_clock, wait_clock):
            gc = tick_clock.global_clock
            n = len(gc)
            for i in range(n):
                if gc[i] > 0:
                    vc = VectorClock([0] * n)
                    vc.require_at_least(i, gc[i])
                    nop = self.nc.sync.nop(nofuse=True)
                    wait_clock.add_sem_waits(nop.ins, ScopedClock({None: vc}))
            self.nc.sync.drain()
            self.nc.all_engine_barrier()
            assert self.sems is not None
            popped = self.nc._tile_sem_poison_stack.pop()
            assert popped is self._sem_poison
            self.nc.clear_and_free_semaphores(
                list(self.sems.allocated().values())
            )
            self.nc.all_engine_barrier()

    return TC


def _split_excess_waits(nc, max_waits=2):
    """Walrus in this container caps sem waits per instruction; hoist any
    excess waits onto fresh same-engine NOPs inserted just before."""
    import concourse.mybir as mybir

    n = 0
    for f in nc.m.functions:
        for bb in f.blocks:
            insts = bb.instructions
            out = []
            for inst in insts:
                si = inst.sync_info
                if si is not None and len(si.on_wait) > max_waits:
                    w = list(si.on_wait)
                    excess, keep = w[:-max_waits], w[-max_waits:]
                    for k in range(0, len(excess), max_waits):
                        nop = mybir.InstNoOp(
                            name=f"I-splitw-{n}", ins=[], outs=[]
                        )
                        n += 1
                        nop.engine = inst.engine
                        nop.sync_info = mybir.SyncInfo(
                            on_wait=excess[k:k + max_waits], on_update=[]
                        )
                        out.append(nop)
                    inst.sync_info = mybir.SyncInfo(
                        on_wait=keep, on_update=si.on_update
                    )
                out.append(inst)
            if n:
                bb.instructions = out
    return nc


def _build():
    import concourse.bass as bass
    import concourse.mybir as mybir

    dt = mybir.dt
    f16, f32 = dt.float16, dt.float32
    AF = mybir.ActivationFunctionType

    nc = bass.Bass()
    xt = nc.declare_dram_parameter("xt", [E, T], f16, isOutput=False)
    wqk = nc.declare_dram_parameter("wqk", [E, 1024], f16, isOutput=False)
    bqk = nc.declare_dram_parameter("bqk", [128, 8], f32, isOutput=False)
    wv = nc.declare_dram_parameter("wv", [E, 512], f16, isOutput=False)
    bv = nc.declare_dram_parameter("bv", [1, 512], f16, isOutput=False)
    wo = nc.declare_dram_parameter("wo", [DL, E], f16, isOutput=False)
    mask = nc.declare_dram_parameter("mask", [128, 128], f16, isOutput=False)
    ones1 = nc.declare_dram_parameter("ones1", [1, 128], f16, isOutput=False)
    out = nc.declare_dram_parameter("out", [T, E], f32, isOutput=True)

    with _make_tc_class()(nc) as tc:
        with (
            tc.tile_pool(name="const", bufs=1) as constp,
            tc.tile_pool(name="xtp", bufs=1) as xtp,
            tc.tile_pool(name="wp", bufs=1) as wp,
            tc.tile_pool(name="qkv", bufs=1) as qkvp,
            tc.tile_pool(name="pt", bufs=6) as ptp,
            tc.tile_pool(name="rec", bufs=2) as recp,
            tc.tile_pool(name="stg", bufs=2) as stgp,
            tc.tile_pool(name="outp", bufs=3) as outp,
            tc.tile_pool(name="psA", bufs=2, space="PSUM") as psA,
            tc.tile_pool(name="psS", bufs=3, space="PSUM") as psS,
            tc.tile_pool(name="psY", bufs=2, space="PSUM") as psY,
            tc.tile_pool(name="psB", bufs=1, space="PSUM") as psB,
        ):
            # ---- constants / weights ----
            bqk_sb = constp.tile([128, 8], f32, tag="bqk")
            nc.sync.dma_start(bqk_sb[:], bqk[:])
            bv_sb = constp.tile([1, 512], f16, tag="bv")
            nc.sync.dma_start(bv_sb[:], bv[:])
            mask_sb = constp.tile([128, 128], f16, tag="mask")
            nc.sync.dma_start(mask_sb[:], mask[:])
            ones_sb = constp.tile([1, 128], f16, tag="ones1")
            nc.sync.dma_start(ones_sb[:], ones1[:])

            xt_sb = []
            for i in range(NE):
                t_ = xtp.tile([128, T], f16, tag=f"xt{i}")
                nc.sync.dma_start(t_[:], xt[i * 128:(i + 1) * 128, :])
                xt_sb.append(t_)
            wqk_sb = []
            for i in range(NE):
                t_ = wp.tile([128, 1024], f16, tag=f"wqk{i}")
                nc.sync.dma_start(t_[:], wqk[i * 128:(i + 1) * 128, :])
                wqk_sb.append(t_)
            wv_sb = []
            for i in range(NE):
                t_ = wp.tile([128, 512], f16, tag=f"wv{i}")
                nc.sync.dma_start(t_[:], wv[i * 128:(i + 1) * 128, :])
                wv_sb.append(t_)
            wo_sb = []
            for i in range(4):
                t_ = wp.tile([128, 1024], f16, tag=f"wo{i}")
                nc.sync.dma_start(t_[:], wo[i * 128:(i + 1) * 128, :])
                wo_sb.append(t_)

            qt_sb = [qkvp.tile([128, T], f16, tag=f"qt{i}", name=f"qt{i}") for i in range(4)]
            kt_sb = [qkvp.tile([128, T], f16, tag=f"kt{i}", name=f"kt{i}") for i in range(4)]
            yt_sb = [qkvp.tile([128, T], f16, tag=f"yt{i}", name=f"yt{i}") for i in range(4)]
            va_sb = [qkvp.tile([128, 8, 65], f16, tag=f"va{i}", name=f"va{i}") for i in range(NS)]

            # t-chunk-outer structure: projections for chunk tcx, then
            # attention for all heads at tcx (keys/values <= tcx are ready),
            # then the output projection for tcx's t-tiles.  The scheduler
            # can interleave across sections to keep the PE stream dense.
            for tcx in range(NT):
                # -- qT/kT projection for this t-chunk --
                for jt in range(8):
                    dest = qt_sb[jt] if jt < 4 else kt_sb[jt - 4]
                    ps = psA.tile([128, 512], f32, tag="psA")
                    for et in range(NE):
                        nc.tensor.matmul(
                            ps[:],
                            wqk_sb[et][:, jt * 128:(jt + 1) * 128],
                            xt_sb[et][:, tcx * 512:(tcx + 1) * 512],
                            start=(et == 0),
                            stop=(et == NE - 1),
                        )
                    nc.scalar.activation(
                        dest[:, tcx * 512:(tcx + 1) * 512], ps[:],
                        AF.Identity, bias=bqk_sb[:, jt:jt + 1],
                    )

                # -- V projection for this chunk's 4 s-tiles --
                for st in range(4 * tcx, 4 * tcx + 4):
                    ps = psA.tile([128, 512], f32, tag="psA")
                    for et in range(NE):
                        nc.tensor.matmul(
                            ps[:],
                            xt_sb[et][:, st * 128:(st + 1) * 128],
                            wv_sb[et][:],
                            start=(et == 0),
                            stop=False,
                        )
                    # bias row: V += 1 * bv
                    nc.tensor.matmul(
                        ps[:], ones_sb[:], bv_sb[:], start=False, stop=True,
                    )
                    va = va_sb[st]
                    nc.vector.tensor_copy(
                        va[:, :, 0:64],
                        ps[:].rearrange("p (h c) -> p h c", c=64),
                    )
                    nc.vector.memset(va[:, :, 64:65], 1.0)

                # -- attention for all heads at this t-chunk --
                nst = 4 * (tcx + 1)
                for h in range(HPC):
                    hp, ho = divmod(h, 2)
                    qrow = slice(ho * 64, (ho + 1) * 64)
                    yps = psY.tile([65, 512], f32, tag="psY")
                    for st in range(nst):
                        sps = psS.tile([128, 512], f32, tag="psS")
                        nc.tensor.matmul(
                            sps[:],
                            kt_sb[hp][qrow, st * 128:(st + 1) * 128],
                            qt_sb[hp][qrow, tcx * 512:(tcx + 1) * 512],
                            start=True,
                            stop=True,
                        )
                        lo = max(0, st * 128 - tcx * 512)
                        pt = ptp.tile([128, 512], f16, tag="pt")
                        nc.scalar.activation(pt[:, lo:512], sps[:, lo:512], AF.Exp)
                        if st * 128 >= tcx * 512:
                            nc.vector.tensor_mul(
                                pt[:, lo:lo + 128], pt[:, lo:lo + 128], mask_sb[:]
                            )
                        nc.tensor.matmul(
                            yps[:, lo:512],
                            va_sb[st][:, h, :],
                            pt[:, lo:512],
                            start=(st == 0),
                            stop=(st == nst - 1),
                        )
                    # divide rows 0..63 by the denominator in row 64
                    rec = recp.tile([1, 512], f16, tag="rec")
                    with nc.allow_low_precision(reason="fp16 softmax recip"):
                        nc.vector.reciprocal(rec[:], yps[64:65, :])
                    bps = psB.tile([64, 512], f32, tag="psB")
                    nc.tensor.matmul(
                        bps[:], ones_sb[:, 0:64], rec[:], start=True, stop=True
                    )
                    bcs = stgp.tile([64, 512], f32, tag="bcs")
                    nc.vector.tensor_copy(bcs[:], bps[:])
                    nc.vector.tensor_mul(
                        yt_sb[hp][qrow, tcx * 512:(tcx + 1) * 512],
                        yps[0:64, :],
                        bcs[:],
                    )

                # -- output projection for this chunk's 4 t-tiles --
                for tt in range(4 * tcx, 4 * tcx + 4):
                    for cc in range(2):
                        ps = psA.tile([128, 512], f32, tag="psA")
                        for hp in range(4):
                            nc.tensor.matmul(
                                ps[:],
                                yt_sb[hp][:, tt * 128:(tt + 1) * 128],
                                wo_sb[hp][:, cc * 512:(cc + 1) * 512],
                                start=(hp == 0),
                                stop=(hp == 3),
                            )
                        osb = outp.tile([128, 512], f32, tag="osb")
                        nc.vector.tensor_copy(osb[:], ps[:])
                        nc.sync.dma_start(
                            out[tt * 128:(tt + 1) * 128,
                                cc * 512:(cc + 1) * 512],
                            osb[:],
                        )
    return _split_excess_waits(nc, max_waits=1)


def _prep_in_maps(x, W_qkv, b_qkv, W_out):
    f16 = np.float16
    x = np.asarray(x, np.float32)
    W_qkv = np.asarray(W_qkv, np.float32)
    b_qkv = np.asarray(b_qkv, np.float32)
    W_out = np.asarray(W_out, np.float32)

    mask = np.triu(np.ones((128, 128), dtype=f16))
    ones1 = np.ones((1, 128), dtype=f16)
    in_maps = []
    for c in range(8):
        b, hg = divmod(c, 2)
        qs = slice(hg * 512, (hg + 1) * 512)
        ks = slice(E + hg * 512, E + (hg + 1) * 512)
        vs = slice(2 * E + hg * 512, 2 * E + (hg + 1) * 512)
        wqk_c = np.concatenate(
            [W_qkv[:, qs] * 0.125, W_qkv[:, ks]], axis=1
        ).astype(f16)
        bqk_c = np.concatenate(
            [b_qkv[qs] * 0.125, b_qkv[ks]]
        ).astype(np.float32).reshape(8, 128).T.copy()
        in_maps.append({
            "xt": np.ascontiguousarray(x[b].T).astype(f16),
            "wqk": wqk_c,
            "bqk": bqk_c,
            "wv": W_qkv[:, vs].astype(f16),
            "bv": b_qkv[vs].astype(f16).reshape(1, 512),
            "wo": W_out[hg * 512:(hg + 1) * 512, :].astype(f16),
            "mask": mask,
            "ones1": ones1,
        })
    return in_maps


def run(x, W_qkv, b_qkv, W_out, b_out, trace=False, **trace_kwargs):
    from concourse.bass_utils import run_bass_kernel_spmd

    if "nc" not in _CACHE:
        _CACHE["nc"] = _build()
    nc = _CACHE["nc"]
    in_maps = _prep_in_maps(x, W_qkv, b_qkv, W_out)
    res = run_bass_kernel_spmd(
        nc, in_maps, list(range(8)), trace=trace, **trace_kwargs
    )
    parts = [r["out"] for r in res.results]
    b_out = np.asarray(b_out, np.float32)
    y = np.stack([parts[2 * b] + parts[2 * b + 1] for b in range(B)]) + b_out
    return y.astype(np.float32), res


def kernel(x, W_qkv, b_qkv, W_out, b_out):
    y, _ = run(x, W_qkv, b_qkv, W_out, b_out, trace=False)
    return y


# revision 24
# speedup vs baseline: 1.0338x; 1.0338x over previous
"""Causal self-attention on 8 trn2 NeuronCores.

Sharding: core c -> (batch b = c//2, head-group hg = c%2 of 8 heads).
Each core computes, for its batch and its 8 heads:
  qT,kT = (x[b] @ Wqk_shard).T        (q pre-scaled by 1/sqrt(hd))
  V     = x[b] @ Wv_shard
  S^T   = kT_h.T @ qT_h  per head     (s on partitions, t on free dim)
  P^T   = exp(S^T) with causal mask   (no max-subtraction: logits are O(5))
  yT    = V_aug.T @ P^T               (V carries a ones column -> row 64 = softmax denom)
  out_partial = y_local @ Wout_rows   ([T, E] fp32 partial sum)
Host: out[b] = partial[2b] + partial[2b+1] + b_out.

All matmul inputs fp16, PSUM accumulation fp32. x is pre-transposed and
pre-cast on host so no on-chip transpose is needed.
"""

import numpy as np

B, T, E, H, HD = 4, 2048, 1024, 16, 64
HPC = 8            # heads per core
DL = HPC * HD      # 512 local y dims per core
NT = T // 512      # 4 t-chunks of 512
NS = T // 128      # 16 s-tiles of 128
NE = E // 128      # 8 e-tiles

_CACHE = {}


def _make_tc_class():
    """TileContext whose tail drain splits sem waits across single-wait NOPs.

    The walrus build in this container rejects instructions carrying more
    than a couple of sync waits ("Too many sync wait commands" on the Tile
    tail Drain), so emit one NOP per logical proc, each with one wait.
    """
    import concourse.tile as tile
    from concourse.vector_clock import ScopedClock, VectorClock

    class TC(tile.TileContext):
        def _drain_and_barrier(self, tick_clock, wait_clock):
            gc = tick_clock.global_clock
            n = len(gc)
            for i in range(n):
                if gc[i] > 0:
                    vc = VectorClock([0] * n)
                    vc.require_at_least(i, gc[i])
                    nop = self.nc.sync.nop(nofuse=True)
                    wait_clock.add_sem_waits(nop.ins, ScopedClock({None: vc}))
            self.nc.sync.drain()
            self.nc.all_engine_barrier()
            assert self.sems is not None
            popped = self.nc._tile_sem_poison_stack.pop()
            assert popped is self._sem_poison
            self.nc.clear_and_free_semaphores(
                list(self.sems.allocated().values())
            )
            self.nc.all_engine_barrier()

    return TC


def _split_excess_waits(nc, max_waits=2):
    """Walrus in this container caps sem waits per instruction; hoist any
    excess waits onto fresh same-engine NOPs inserted just before."""
    import concourse.mybir as mybir

    n = 0
    for f in nc.m.functions:
        for bb in f.blocks:
            insts = bb.instructions
            out = []
            for inst in insts:
                si = inst.sync_info
                if si is not None and len(si.on_wait) > max_waits:
                    w = list(si.on_wait)
                    excess, keep = w[:-max_waits], w[-max_waits:]
                    for k in range(0, len(excess), max_waits):
                        nop = mybir.InstNoOp(
                            name=f"I-splitw-{n}", ins=[], outs=[]
                        )
                        n += 1
                        nop.engine = inst.engine
                        nop.sync_info = mybir.SyncInfo(
                            on_wait=excess[k:k + max_waits], on_update=[]
                        )
                        out.append(nop)
                    inst.sync_info = mybir.SyncInfo(
                        on_wait=keep, on_update=si.on_update
                    )
                out.append(inst)
            if n:
                bb.instructions = out
    return nc


def _build():
    import concourse.bass as bass
    import concourse.mybir as mybir

    dt = mybir.dt
    f16, f32 = dt.float16, dt.float32
    AF = mybir.ActivationFunctionType

    nc = bass.Bass()
    xt = nc.declare_dram_parameter("xt", [E, T], f16, isOutput=False)
    wqk = nc.declare_dram_parameter("wqk", [E, 1024], f16, isOutput=False)
    bqk = nc.declare_dram_parameter("bqk", [128, 8], f32, isOutput=False)
    wv = nc.declare_dram_parameter("wv", [E, 512], f16, isOutput=False)
    bv = nc.declare_dram_parameter("bv", [1, 512], f16, isOutput=False)
    wo = nc.declare_dram_parameter("wo", [DL, E], f16, isOutput=False)
    mask = nc.declare_dram_parameter("mask", [128, 128], f16, isOutput=False)
    ones1 = nc.declare_dram_parameter("ones1", [1, 128], f16, isOutput=False)
    # onehot[h] block: [8, 64] with row h all-ones (recip broadcast selector)
    onehot = nc.declare_dram_parameter("onehot", [8, 512], f16, isOutput=False)
    out = nc.declare_dram_parameter("out", [T, E], f32, isOutput=True)

    with _make_tc_class()(nc) as tc:
        with (
            tc.tile_pool(name="const", bufs=1) as constp,
            tc.tile_pool(name="xtp", bufs=1) as xtp,
            tc.tile_pool(name="wp", bufs=1) as wp,
            tc.tile_pool(name="qkv", bufs=1) as qkvp,
            tc.tile_pool(name="pt", bufs=6) as ptp,
            tc.tile_pool(name="rec", bufs=2) as recp,
            tc.tile_pool(name="stg", bufs=2) as stgp,
            tc.tile_pool(name="outp", bufs=3) as outp,
            tc.tile_pool(name="psA", bufs=2, space="PSUM") as psA,
            tc.tile_pool(name="psS", bufs=4, space="PSUM") as psS,
            tc.tile_pool(name="psY", bufs=2, space="PSUM") as psY,
        ):
            # ---- constants / weights ----
            bqk_sb = constp.tile([128, 8], f32, tag="bqk")
            nc.sync.dma_start(bqk_sb[:], bqk[:])
            bv_sb = constp.tile([1, 512], f16, tag="bv")
            nc.sync.dma_start(bv_sb[:], bv[:])
            mask_sb = constp.tile([128, 128], f16, tag="mask")
            nc.sync.dma_start(mask_sb[:], mask[:])
            ones_sb = constp.tile([1, 128], f16, tag="ones1")
            nc.sync.dma_start(ones_sb[:], ones1[:])
            onehot_sb = constp.tile([8, 512], f16, tag="onehot")
            nc.sync.dma_start(onehot_sb[:], onehot[:])

            xt_sb = []
            for i in range(NE):
                t_ = xtp.tile([128, T], f16, tag=f"xt{i}")
                nc.sync.dma_start(t_[:], xt[i * 128:(i + 1) * 128, :])
                xt_sb.append(t_)
            wqk_sb = []
            for i in range(NE):
                t_ = wp.tile([128, 1024], f16, tag=f"wqk{i}")
                nc.sync.dma_start(t_[:], wqk[i * 128:(i + 1) * 128, :])
                wqk_sb.append(t_)
            wv_sb = []
            for i in range(NE):
                t_ = wp.tile([128, 512], f16, tag=f"wv{i}")
                nc.sync.dma_start(t_[:], wv[i * 128:(i + 1) * 128, :])
                wv_sb.append(t_)
            wo_sb = []
            for i in range(4):
                t_ = wp.tile([128, 1024], f16, tag=f"wo{i}")
                nc.sync.dma_start(t_[:], wo[i * 128:(i + 1) * 128, :])
                wo_sb.append(t_)

            qt_sb = [qkvp.tile([128, T], f16, tag=f"qt{i}", name=f"qt{i}") for i in range(4)]
            kt_sb = [qkvp.tile([128, T], f16, tag=f"kt{i}", name=f"kt{i}") for i in range(4)]
            yt_sb = [qkvp.tile([128, T], f16, tag=f"yt{i}", name=f"yt{i}") for i in range(4)]
            va_sb = [qkvp.tile([128, 8, 65], f16, tag=f"va{i}", name=f"va{i}") for i in range(NS)]

            # t-chunk-outer structure: projections for chunk tcx, then
            # attention for all heads at tcx (keys/values <= tcx are ready),
            # then the output projection for tcx's t-tiles.  The scheduler
            # can interleave across sections to keep the PE stream dense.
            for tcx in range(NT):
                # -- qT/kT projection for this t-chunk --
                for jt in range(8):
                    dest = qt_sb[jt] if jt < 4 else kt_sb[jt - 4]
                    ps = psA.tile([128, 512], f32, tag="psA")
                    for et in range(NE):
                        nc.tensor.matmul(
                            ps[:],
                            wqk_sb[et][:, jt * 128:(jt + 1) * 128],
                            xt_sb[et][:, tcx * 512:(tcx + 1) * 512],
                            start=(et == 0),
                            stop=(et == NE - 1),
                        )
                    nc.vector.tensor_scalar_add(
                        dest[:, tcx * 512:(tcx + 1) * 512], ps[:],
                        bqk_sb[:, jt:jt + 1],
                    )

                # -- V projection for this chunk's 4 s-tiles --
                for st in range(4 * tcx, 4 * tcx + 4):
                    ps = psA.tile([128, 512], f32, tag="psA")
                    for et in range(NE):
                        nc.tensor.matmul(
                            ps[:],
                            xt_sb[et][:, st * 128:(st + 1) * 128],
                            wv_sb[et][:],
                            start=(et == 0),
                            stop=False,
                        )
                    # bias row: V += 1 * bv
                    nc.tensor.matmul(
                        ps[:], ones_sb[:], bv_sb[:], start=False, stop=True,
                    )
                    va = va_sb[st]
                    nc.vector.tensor_copy(
                        va[:, :, 0:64],
                        ps[:].rearrange("p (h c) -> p h c", c=64),
                    )
                    nc.vector.memset(va[:, :, 64:65], 1.0)

                # -- attention for all heads at this t-chunk --
                nst = 4 * (tcx + 1)
                for h in range(HPC):
                    hp, ho = divmod(h, 2)
                    qrow = slice(ho * 64, (ho + 1) * 64)
                    yps = psY.tile([65, 512], f32, tag="psY")
                    for g in range(0, nst, 4):
                        gn = min(4, nst - g)
                        sgrp = []
                        for st in range(g, g + gn):
                            sps = psS.tile([128, 512], f32, tag="psS")
                            nc.tensor.matmul(
                                sps[:],
                                kt_sb[hp][qrow, st * 128:(st + 1) * 128],
                                qt_sb[hp][qrow, tcx * 512:(tcx + 1) * 512],
                                start=True,
                                stop=True,
                            )
                            lo = max(0, st * 128 - tcx * 512)
                            pt = ptp.tile([128, 512], f16, tag="pt")
                            nc.scalar.activation(
                                pt[:, lo:512], sps[:, lo:512], AF.Exp
                            )
                            if st * 128 >= tcx * 512:
                                nc.vector.tensor_mul(
                                    pt[:, lo:lo + 128], pt[:, lo:lo + 128],
                                    mask_sb[:],
                                )
                            sgrp.append((st, lo, pt))
                        for st, lo, pt in sgrp:
                            nc.tensor.matmul(
                                yps[:, lo:512],
                                va_sb[st][:, h, :],
                                pt[:, lo:512],
                                start=(st == 0),
                                stop=(st == nst - 1),
                            )
                    # divide rows 0..63 by the denominator in row 64
                    rec = recp.tile([1, 512], f16, tag="rec")
                    with nc.allow_low_precision(reason="fp16 softmax recip"):
                        nc.vector.reciprocal(rec[:], yps[64:65, :])
                    bps = psA.tile([64, 512], f32, tag="psA")
                    nc.tensor.matmul(
                        bps[:], ones_sb[:, 0:64], rec[:], start=True, stop=True
                    )
                    bcs = stgp.tile([64, 512], f32, tag="bcs", bufs=3)
                    nc.vector.tensor_copy(bcs[:], bps[:])
                    nc.vector.tensor_mul(
                        yt_sb[hp][qrow, tcx * 512:(tcx + 1) * 512],
                        yps[0:64, :],
                        bcs[:],
                    )

                # -- output projection for this chunk's 4 t-tiles --
                for tt in range(4 * tcx, 4 * tcx + 4):
                    for cc in range(2):
                        ps = psA.tile([128, 512], f32, tag="psA")
                        for hp in range(4):
                            nc.tensor.matmul(
                                ps[:],
                                yt_sb[hp][:, tt * 128:(tt + 1) * 128],
                                wo_sb[hp][:, cc * 512:(cc + 1) * 512],
                                start=(hp == 0),
                                stop=(hp == 3),
                            )
                        osb = outp.tile([128, 512], f32, tag="osb")
                        nc.vector.tensor_copy(osb[:], ps[:])
                        nc.sync.dma_start(
                            out[tt * 128:(tt + 1) * 128,
                                cc * 512:(cc + 1) * 512],
                            osb[:],
                        )
    return _split_excess_waits(nc, max_waits=1)


def _prep_in_maps(x, W_qkv, b_qkv, W_out):
    f16 = np.float16
    x = np.asarray(x, np.float32)
    W_qkv = np.asarray(W_qkv, np.float32)
    b_qkv = np.asarray(b_qkv, np.float32)
    W_out = np.asarray(W_out, np.float32)

    mask = np.triu(np.ones((128, 128), dtype=f16))
    ones1 = np.ones((1, 128), dtype=f16)
    in_maps = []
    for c in range(8):
        b, hg = divmod(c, 2)
        qs = slice(hg * 512, (hg + 1) * 512)
        ks = slice(E + hg * 512, E + (hg + 1) * 512)
        vs = slice(2 * E + hg * 512, 2 * E + (hg + 1) * 512)
        wqk_c = np.concatenate(
            [W_qkv[:, qs] * 0.125, W_qkv[:, ks]], axis=1
        ).astype(f16)
        bqk_c = np.concatenate(
            [b_qkv[qs] * 0.125, b_qkv[ks]]
        ).astype(np.float32).reshape(8, 128).T.copy()
        onehot = np.zeros((8, 512), dtype=f16)
        for h in range(8):
            onehot[h, h * 64:(h + 1) * 64] = 1.0
        in_maps.append({
            "xt": np.ascontiguousarray(x[b].T).astype(f16),
            "onehot": onehot,
            "wqk": wqk_c,
            "bqk": bqk_c,
            "wv": W_qkv[:, vs].astype(f16),
            "bv": b_qkv[vs].astype(f16).reshape(1, 512),
            "wo": W_out[hg * 512:(hg + 1) * 512, :].astype(f16),
            "mask": mask,
            "ones1": ones1,
        })
    return in_maps


def run(x, W_qkv, b_qkv, W_out, b_out, trace=False, **trace_kwargs):
    from concourse.bass_utils import run_bass_kernel_spmd

    if "nc" not in _CACHE:
        _CACHE["nc"] = _build()
    nc = _CACHE["nc"]
    in_maps = _prep_in_maps(x, W_qkv, b_qkv, W_out)
    res = run_bass_kernel_spmd(
        nc, in_maps, list(range(8)), trace=trace, **trace_kwargs
    )
    parts = [r["out"] for r in res.results]
    b_out = np.asarray(b_out, np.float32)
    y = np.stack([parts[2 * b] + parts[2 * b + 1] for b in range(B)]) + b_out
    return y.astype(np.float32), res


def kernel(x, W_qkv, b_qkv, W_out, b_out):
    y, _ = run(x, W_qkv, b_qkv, W_out, b_out, trace=False)
    return y
